# revision 1
# baseline (speedup 1.0000x reference)
"""Trainium2 Bass kernel: CANE FeatureEmbedding GNN message passing.

Strategy (node-range sharding, 8 cores):
  - Nodes are range-partitioned: core r owns nodes [r*6250, (r+1)*6250).
  - Edges are assigned to the core owning their DESTINATION (row = edge_index[1]).
  - Algebraic collapse of the peer branch (gather and scatter both use `row`):
        h_peer[i] = relu( sqrt(deg_i)*(W_px x_i + b_p) + deg_i^-1/2 * (W_pe S_ea[i]) )
    where S_ea[i] = segment_sum(edge_attrs, row). This removes the [E,160]x[160,96]
    per-edge MLP entirely.
  - Per-edge work that remains: h_e = relu(ea @ W_edge.T + b_edge), segment sums of
    ea and h_e over row, and M[i] = sum_{e: row=i} h_e_agg[col[e]].
  - Segment sums use a fixed "slot grid": each node gets C slots; edge k of node v
    goes to (block b = v//128, partition v%128, slot k). One matmul per slot tile
    accumulates S_ea directly in PSUM; h_e slots land in per-slot PSUM columns and
    are reduced after a single big ReLU. Nodes with deg > C spill to per-block
    overflow tiles handled with one-hot scatter matmuls.
  - One AllGather shares h_e_agg; M is built from an indirect-DMA gather of
    h_e_agg[col] in the same slot layout, then reduced along slots.
"""

import numpy as np

import concourse.bass as bass
import concourse.mybir as mybir
import concourse.tile as tile
from concourse import bacc
from concourse._compat import get_trn_type
from concourse.bass import IndirectOffsetOnAxis
from concourse.bass_utils import run_bass_kernel_spmd

F32 = mybir.dt.float32
I32 = mybir.dt.int32
AX = mybir.AxisListType
OP = mybir.AluOpType
ACT = mybir.ActivationFunctionType


class Cfg:
    def __init__(self, N=50000, E=800000, ncores=8, C=15, t_ov=3):
        self.N = N
        self.E = E
        self.ncores = ncores
        self.NPC = N // ncores            # nodes per core
        self.NBLK = (self.NPC + 127) // 128
        self.PADN = self.NBLK * 128       # padded nodes per core
        self.C = C                        # grid slots per node
        self.t_ov = t_ov                  # overflow tiles per block
        self.TPB = C + t_ov               # tiles per block
        self.ND = 128                     # node feature dim
        self.ED = 32                      # edge feature dim
        self.OUTD = 224

    def key(self):
        return (self.N, self.E, self.ncores, self.C, self.t_ov)


def build_program(cfg, skip=()):
    """Build the SPMD Bass program (same NEFF for all cores).

    skip: iterable of {"gather", "slotmm", "overflow", "phasec", "collective"} —
    timing-experiment knobs only (results are wrong when used)."""
    skip = set(skip)
    nc = bacc.Bacc(get_trn_type() or "TRN2", target_bir_lowering=False, debug=True)
    NBLK, TPB, C, t_ov, PADN = cfg.NBLK, cfg.TPB, cfg.C, cfg.t_ov, cfg.PADN
    TOT = NBLK * TPB

    eat = nc.declare_dram_parameter("eat", [33, TOT * 128], F32, isOutput=False)
    gidx = nc.declare_dram_parameter("gidx", [128, NBLK * TPB], I32, isOutput=False)
    rlov = nc.declare_dram_parameter("rlov", [128, NBLK * t_ov], F32, isOutput=False)
    xt = nc.declare_dram_parameter("xt", [128, PADN], F32, isOutput=False)
    dinvp = nc.declare_dram_parameter("dinv", [128, NBLK], F32, isOutput=False)
    sqdp = nc.declare_dram_parameter("sqd", [128, NBLK], F32, isOutput=False)
    sdrow = nc.declare_dram_parameter("sdrow", [1, PADN], F32, isOutput=False)
    rconst = nc.declare_dram_parameter("rconst", [33, 64], F32, isOutput=False)
    wegot = nc.declare_dram_parameter("wegot", [128, 64], F32, isOutput=False)
    wpxt = nc.declare_dram_parameter("wpxt", [128, 96], F32, isOutput=False)
    wpet = nc.declare_dram_parameter("wpet", [32, 96], F32, isOutput=False)
    bego = nc.declare_dram_parameter("bego", [1, 64], F32, isOutput=False)
    bpeer = nc.declare_dram_parameter("bpeer", [1, 96], F32, isOutput=False)
    iota = nc.declare_dram_parameter("iota", [128, 128], F32, isOutput=False)
    ident = nc.declare_dram_parameter("ident", [128, 128], F32, isOutput=False)
    outp = nc.declare_dram_parameter("out", [PADN, cfg.OUTD], F32, isOutput=True)

    with tile.TileContext(nc) as tc:
        with (
            tc.tile_pool(name="const", bufs=1) as cp,
            tc.tile_pool(name="resident", bufs=1) as rp,
            tc.tile_pool(name="dram", bufs=1, space="DRAM") as dp,
        ):
            rc = cp.tile([33, 64], F32)
            nc.sync.dma_start(rc[:], rconst[:])
            wego_sb = cp.tile([128, 64], F32)
            nc.sync.dma_start(wego_sb[:], wegot[:])
            wpx_sb = cp.tile([128, 96], F32)
            nc.sync.dma_start(wpx_sb[:], wpxt[:])
            wpe_sb = cp.tile([32, 96], F32)
            nc.sync.dma_start(wpe_sb[:], wpet[:])
            bego_sb = cp.tile([1, 64], F32)
            nc.sync.dma_start(bego_sb[:], bego[:])
            bpeer_sb = cp.tile([1, 96], F32)
            nc.sync.dma_start(bpeer_sb[:], bpeer[:])
            iota_sb = cp.tile([128, 128], F32)
            nc.sync.dma_start(iota_sb[:], iota[:])
            ident_sb = cp.tile([128, 128], F32)
            nc.sync.dma_start(ident_sb[:], ident[:])
            ones_sb = cp.tile([1, 128], F32)
            nc.gpsimd.memset(ones_sb[:], 1.0)

            xt_sb = rp.tile([128, PADN], F32)
            nc.sync.dma_start(xt_sb[:], xt[:])
            gidx_sb = rp.tile([128, NBLK * TPB], I32)
            nc.sync.dma_start(gidx_sb[:], gidx[:])
            rlov_sb = rp.tile([128, NBLK * t_ov], F32)
            nc.sync.dma_start(rlov_sb[:], rlov[:])
            dinv_sb = rp.tile([128, NBLK], F32)
            nc.sync.dma_start(dinv_sb[:], dinvp[:])
            sqd_sb = rp.tile([128, NBLK], F32)
            nc.sync.dma_start(sqd_sb[:], sqdp[:])
            sdrow_sb = rp.tile([1, PADN], F32)
            nc.sync.dma_start(sdrow_sb[:], sdrow[:])

            sea_sb = rp.tile([128, NBLK * 32], F32)
            heagg_sb = rp.tile([128, NBLK * 32], F32)

            zloc = dp.tile([PADN, 32], F32)
            zag = dp.tile([cfg.ncores * PADN, 32], F32, addr_space="Shared")

            # ---------------- Phase A: per-edge MLP + segment sums ----------
            with (
                tc.tile_pool(name="eatp", bufs=int(getattr(cfg, "eat_bufs", 3))) as eatp,
                tc.tile_pool(name="workA", bufs=int(getattr(cfg, "wa_bufs", 4))) as wp,
                tc.tile_pool(name="psA", bufs=2, space="PSUM") as psA,
                tc.tile_pool(name="psB", bufs=int(getattr(cfg, "psb_bufs", 2)), space="PSUM") as psB,
                tc.tile_pool(name="psP", bufs=int(getattr(cfg, "psp_bufs", 2)), space="PSUM") as psP,
                tc.tile_pool(name="psO", bufs=2, space="PSUM") as psO,
            ):
                for b in range(NBLK):
                    ech = eatp.tile([33, TPB * 128], F32, tag="ech")
                    nc.sync.dma_start(ech[:], eat[:, b * TPB * 128:(b + 1) * TPB * 128])
                    bankA = psA.tile([128, 32], F32, tag="bankA")
                    bankB = psB.tile([128, C * 32], F32, tag="bankB")
                    for j in range(C):
                        if "slotmm" in skip:
                            break
                        lh = ech[:, j * 128:(j + 1) * 128]
                        nc.tensor.matmul(bankA[:], lh, rc[:, 0:32],
                                         start=(j == 0), stop=(j == C - 1))
                        nc.tensor.matmul(bankB[:, j * 32:(j + 1) * 32], lh, rc[:, 32:64],
                                         start=(j == 0), stop=(j == C - 1))
                    bankO = psO.tile([128, 64], F32, tag="bankO")
                    for o in ([] if "overflow" in skip else range(t_ov)):
                        lh = ech[:, (C + o) * 128:(C + o + 1) * 128]
                        pc = psP.tile([128, 64], F32, tag="pc")
                        nc.tensor.matmul(pc[:], lh, rc[:, 0:64], start=True, stop=True)
                        ov = wp.tile([128, 64], F32, tag="ov")
                        nc.vector.tensor_copy(ov[:, 0:32], pc[:, 0:32])
                        nc.vector.tensor_scalar_max(ov[:, 32:64], pc[:, 32:64], 0.0)
                        oh = wp.tile([128, 128], F32, tag="oh")
                        k = b * t_ov + o
                        nc.vector.tensor_scalar(
                            out=oh[:], in0=iota_sb[:],
                            scalar1=rlov_sb[:, k:k + 1], scalar2=None,
                            op0=OP.is_equal,
                        )
                        nc.tensor.matmul(bankO[:], oh[:], ov[:],
                                         start=(o == 0), stop=(o == t_ov - 1))
                    relu_st = wp.tile([128, C * 32], F32, tag="relu")
                    nc.scalar.activation(relu_st[:], bankB[:], ACT.Relu)
                    t_he = wp.tile([128, 32], F32, tag="the")
                    nc.vector.tensor_reduce(
                        t_he[:],
                        relu_st[:].rearrange("p (j c) -> p c j", j=C),
                        axis=AX.X, op=OP.add,
                    )
                    ovsb = wp.tile([128, 64], F32, tag="ovsb")
                    nc.vector.tensor_copy(ovsb[:], bankO[:])
                    nc.vector.tensor_tensor(
                        out=heagg_sb[:, b * 32:(b + 1) * 32],
                        in0=t_he[:], in1=ovsb[:, 32:64], op=OP.add)
                    nc.vector.tensor_tensor(
                        out=sea_sb[:, b * 32:(b + 1) * 32],
                        in0=ovsb[:, 0:32], in1=bankA[:], op=OP.add)

            # h_e_agg -> DRAM -> AllGather
            nc.sync.dma_start(
                zloc[:].rearrange("(b p) c -> p b c", p=128),
                heagg_sb[:].rearrange("p (b c) -> p b c", c=32),
            )
            if "collective" not in skip:
                for _rep in range(int(getattr(cfg, "ag_rep", 1))):
                    nc.gpsimd.collective_compute(
                        "AllGather", OP.bypass,
                        ins=[zloc.opt()], outs=[zag.opt()],
                        replica_groups=[list(range(cfg.ncores))],
                    )

            # ------------- Phase B+C: gather/M + node-level MLPs ------------
            with (
                tc.tile_pool(name="workB", bufs=int(getattr(cfg, "wb_bufs", 4))) as wb,
                tc.tile_pool(name="outp_pool", bufs=int(getattr(cfg, "op_bufs", 3))) as op_pool,
                tc.tile_pool(name="psM", bufs=2, space="PSUM") as psM,
                tc.tile_pool(name="ps1", bufs=2, space="PSUM") as ps1,
                tc.tile_pool(name="ps2", bufs=2, space="PSUM") as ps2,
                tc.tile_pool(name="ps3", bufs=1, space="PSUM") as ps3,
                tc.tile_pool(name="psT", bufs=1, space="PSUM") as psT,
            ):
                for b in range(NBLK):
                    g = wb.tile([128, TPB * 32], F32, tag="g", bufs=8)
                    for t in ([] if "gather" in skip else range(TPB)):
                        for _rep in range(int(getattr(cfg, "gather_rep", 1))):
                            nc.gpsimd.indirect_dma_start(
                                out=g[:, t * 32:(t + 1) * 32],
                                out_offset=None,
                                in_=zag[:],
                                in_offset=IndirectOffsetOnAxis(
                                    ap=gidx_sb[:, b * TPB + t:b * TPB + t + 1], axis=0),
                            )
                    m_main = wb.tile([128, 32], F32, tag="mmain")
                    nc.vector.tensor_reduce(
                        m_main[:],
                        g[:, 0:C * 32].rearrange("p (j c) -> p c j", j=C),
                        axis=AX.X, op=OP.add,
                    )
                    pm = psM.tile([128, 32], F32, tag="pm")
                    for o in range(t_ov):
                        oh = wb.tile([128, 128], F32, tag="oh2")
                        k = b * t_ov + o
                        nc.vector.tensor_scalar(
                            out=oh[:], in0=iota_sb[:],
                            scalar1=rlov_sb[:, k:k + 1], scalar2=None,
                            op0=OP.is_equal,
                        )
                        nc.tensor.matmul(pm[:], oh[:], g[:, (C + o) * 32:(C + o + 1) * 32],
                                         start=(o == 0), stop=(o == t_ov - 1))
                    outst = op_pool.tile([128, cfg.OUTD], F32, tag="outst")
                    t_m = wb.tile([128, 32], F32, tag="tm")
                    nc.vector.tensor_tensor(out=t_m[:], in0=m_main[:], in1=pm[:], op=OP.add)
                    nc.vector.tensor_scalar_mul(outst[:, 96:128], t_m[:], dinv_sb[:, b:b + 1])
                    nc.vector.tensor_copy(outst[:, 64:96], heagg_sb[:, b * 32:(b + 1) * 32])

                    # h_ego = relu(x W_ego^T + b_ego)
                    if "phasec" in skip:
                        nc.sync.dma_start(outp[b * 128:(b + 1) * 128, :], outst[:])
                        continue
                    p1 = ps1.tile([128, 64], F32, tag="p1")
                    nc.tensor.matmul(p1[:], ones_sb[:], bego_sb[:], start=True, stop=False)
                    nc.tensor.matmul(p1[:], xt_sb[:, b * 128:(b + 1) * 128], wego_sb[:],
                                     start=False, stop=True)
                    nc.vector.tensor_scalar_max(outst[:, 0:64], p1[:], 0.0)

                    # h_peer = relu(sqd*(W_px x) + sqd*b_p + W_pe (dinv*S_ea))
                    p2 = ps2.tile([128, 96], F32, tag="p2")
                    nc.tensor.matmul(p2[:], xt_sb[:, b * 128:(b + 1) * 128], wpx_sb[:],
                                     start=True, stop=True)
                    p3 = ps3.tile([128, 96], F32, tag="p3")
                    nc.tensor.matmul(p3[:], sdrow_sb[:, b * 128:(b + 1) * 128], bpeer_sb[:],
                                     start=True, stop=False)
                    t_s = wb.tile([128, 32], F32, tag="ts")
                    nc.vector.tensor_scalar_mul(t_s[:], sea_sb[:, b * 32:(b + 1) * 32],
                                                dinv_sb[:, b:b + 1])
                    pt = psT.tile([32, 128], F32, tag="pt")
                    nc.tensor.matmul(pt[:], t_s[:], ident_sb[:], is_transpose=True,
                                     start=True, stop=True)
                    seat = wb.tile([32, 128], F32, tag="seat")
                    nc.vector.tensor_copy(seat[:], pt[:])
                    nc.tensor.matmul(p3[:], seat[:], wpe_sb[:], start=False, stop=True)
                    t_u = wb.tile([128, 96], F32, tag="tu")
                    nc.vector.tensor_scalar_mul(t_u[:], p2[:], sqd_sb[:, b:b + 1])
                    nc.vector.tensor_tensor(out=t_u[:], in0=t_u[:], in1=p3[:], op=OP.add)
                    nc.vector.tensor_scalar_max(outst[:, 128:224], t_u[:], 0.0)

                    nc.sync.dma_start(outp[b * 128:(b + 1) * 128, :], outst[:])
    nc.compile()
    return nc


def host_prep(cfg, x, edge_attrs, edge_index):
    """Shard + lay out inputs for the slot-grid kernel. Pure index work + O(N)
    scalar prep (degree normalizers); all O(E*H)/O(N*H) FP math runs on device."""
    N, E, C, NBLK, TPB, t_ov, NPC, PADN = (cfg.N, cfg.E, cfg.C, cfg.NBLK,
                                           cfg.TPB, cfg.t_ov, cfg.NPC, cfg.PADN)
    row = np.asarray(edge_index[1]).astype(np.int64)
    col = np.asarray(edge_index[0]).astype(np.int64)
    ea = np.asarray(edge_attrs, dtype=np.float32)
    xf = np.asarray(x, dtype=np.float32)

    deg = np.bincount(row, minlength=N)
    degf = np.maximum(deg, 1).astype(np.float64)
    dinv = np.where(deg > 0, degf ** -0.5, 0.0).astype(np.float32)
    sqd = np.sqrt(deg.astype(np.float64)).astype(np.float32)

    core = row // NPC
    lrow = row - core * NPC
    blk = lrow // 128
    part = lrow % 128

    # rank of each edge within its destination node
    order = np.argsort(row, kind="stable")
    sorted_row = row[order]
    starts = np.searchsorted(sorted_row, np.arange(N), side="left")
    rank = np.empty(E, np.int64)
    rank[order] = np.arange(E) - starts[sorted_row]

    is_grid = rank < C
    ovsel = ~is_grid
    ove = np.where(ovsel)[0]
    ovkey = core[ove] * NBLK + blk[ove]
    o_order = np.argsort(ovkey, kind="stable")
    ove = ove[o_order]
    okey_sorted = ovkey[o_order]
    ostarts = np.searchsorted(okey_sorted, np.arange(NBLK * cfg.ncores), side="left")
    opos = np.arange(ove.size) - ostarts[okey_sorted]
    otile = C + opos // 128
    opart = opos % 128
    if ove.size and otile.max() >= TPB:
        raise ValueError("overflow tiles exceeded; raise t_ov")

    # tile index + within-tile partition for every edge
    tile_idx = np.empty(E, np.int64)
    tpart = np.empty(E, np.int64)
    ge = np.where(is_grid)[0]
    tile_idx[ge] = blk[ge] * TPB + rank[ge]
    tpart[ge] = part[ge]
    tile_idx[ove] = blk[ove] * TPB + otile
    tpart[ove] = opart

    zrow = (col // NPC) * PADN + (col % NPC)     # row in allgathered z table
    assert NPC < PADN, "pad-slot gathers need a guaranteed-zero dummy row"
    ZPAD = PADN - 1                               # core0 dummy node -> zeros

    TOTC = NBLK * TPB
    in_maps = []
    # constants shared by all cores are built once
    consts = None
    for r in range(cfg.ncores):
        sel = core == r
        e_idx = np.where(sel)[0]
        t_i = tile_idx[e_idx]
        t_p = tpart[e_idx]
        colpos = t_i * 128 + t_p

        EAT = np.zeros((33, TOTC * 128), np.float32)
        EAT[:32, colpos] = ea[e_idx].T
        EAT[32, colpos] = 1.0

        GIDX = np.full((128, TOTC), ZPAD, np.int32)
        GIDX[t_p, t_i] = zrow[e_idx].astype(np.int32)

        RLOV = np.full((128, NBLK * t_ov), 200.0, np.float32)
        ovm = sel[ove] if False else None
        ov_r = ove[core[ove] == r]
        op_r = opart[core[ove] == r]
        ot_r = otile[core[ove] == r]
        ob_r = blk[ov_r]
        RLOV[op_r, ob_r * t_ov + (ot_r - C)] = part[ov_r].astype(np.float32)

        lo, hi = r * NPC, (r + 1) * NPC
        XT = np.zeros((128, PADN), np.float32)
        XT[:, :NPC] = xf[lo:hi].T
        dl = np.zeros(PADN, np.float32)
        dl[:NPC] = dinv[lo:hi]
        sl = np.zeros(PADN, np.float32)
        sl[:NPC] = sqd[lo:hi]
        DINV = dl.reshape(NBLK, 128).T.copy()
        SQD = sl.reshape(NBLK, 128).T.copy()
        SDROW = sl.reshape(1, PADN)

        m = {
            "eat": EAT, "gidx": GIDX, "rlov": RLOV, "xt": XT,
            "dinv": DINV, "sqd": SQD, "sdrow": SDROW,
        }
        in_maps.append(m)
    return in_maps


def make_consts(cfg, W_peer, b_peer, W_ego, b_ego, W_edge, b_edge):
    RCONST = np.zeros((33, 64), np.float32)
    RCONST[:32, :32] = np.eye(32, dtype=np.float32)
    RCONST[:32, 32:64] = np.asarray(W_edge, np.float32).T
    RCONST[32, 32:64] = np.asarray(b_edge, np.float32)
    consts = {
        "rconst": RCONST,
        "wegot": np.ascontiguousarray(np.asarray(W_ego, np.float32).T),
        "wpxt": np.ascontiguousarray(np.asarray(W_peer, np.float32)[:, :128].T),
        "wpet": np.ascontiguousarray(np.asarray(W_peer, np.float32)[:, 128:].T),
        "bego": np.asarray(b_ego, np.float32).reshape(1, 64),
        "bpeer": np.asarray(b_peer, np.float32).reshape(1, 96),
        "iota": np.broadcast_to(np.arange(128, dtype=np.float32), (128, 128)).copy(),
        "ident": np.eye(128, dtype=np.float32),
    }
    return consts


_CACHE = {}
RUN_KWARGS = {}


def kernel(x, edge_attrs, W_peer, b_peer, W_ego, b_ego, W_edge, b_edge, edge_index):
    x = np.asarray(x)
    edge_attrs = np.asarray(edge_attrs)
    edge_index = np.asarray(edge_index)
    N, E = x.shape[0], edge_attrs.shape[0]

    # pick t_ov from the actual degree distribution (>=3 keeps NEFF cache warm
    # for the expected data)
    row = edge_index[1].astype(np.int64)
    C = 15
    ncores = 8
    NPC = N // ncores
    NBLK = (NPC + 127) // 128
    deg = np.bincount(row, minlength=N)
    ovn = np.maximum(deg - C, 0)
    nodes = np.arange(N)
    bkey = (nodes // NPC) * NBLK + (nodes % NPC) // 128
    ovblk = np.bincount(bkey, weights=ovn.astype(np.float64), minlength=NBLK * ncores)
    t_ov = max(3, int(np.ceil(ovblk.max() / 128.0)))

    cfg = Cfg(N=N, E=E, ncores=ncores, C=C, t_ov=t_ov)
    key = cfg.key()
    if key not in _CACHE:
        _CACHE[key] = build_program(cfg)
    nc = _CACHE[key]

    in_maps = host_prep(cfg, x, edge_attrs, edge_index)
    consts = make_consts(cfg, W_peer, b_peer, W_ego, b_ego, W_edge, b_edge)
    for m in in_maps:
        m.update(consts)

    res = run_bass_kernel_spmd(nc, in_maps, core_ids=list(range(cfg.ncores)),
                               **RUN_KWARGS)
    out = np.empty((N, cfg.OUTD), np.float32)
    for r in range(cfg.ncores):
        out[r * cfg.NPC:(r + 1) * cfg.NPC] = res.results[r]["out"][:cfg.NPC]
    if RUN_KWARGS:
        kernel.last_result = res
    return out



# revision 28
# speedup vs baseline: 1.1538x; 1.1538x over previous
"""Trainium2 Bass kernel (fp16 compute): CANE FeatureEmbedding GNN message passing.

Strategy (node-range sharding, 8 cores):
  - Nodes are range-partitioned: core r owns nodes [r*6250, (r+1)*6250).
  - Edges are assigned to the core owning their DESTINATION (row = edge_index[1]).
  - Algebraic collapse of the peer branch (gather and scatter both use `row`):
        h_peer[i] = relu( (sqd_i x_i) W_px^T + sqd_i b_p + (dinv_i S_ea[i]) W_pe^T )
    where S_ea[i] = segment_sum(edge_attrs, row), sqd = sqrt(deg), dinv = deg^-1/2.
    This removes the [E,160]x[160,96] per-edge MLP entirely; sqd*x is prescaled
    on the host.
  - Per-edge work that remains: h_e = relu(ea @ W_edge.T + b_edge), segment sums
    of ea and h_e over row, and M[i] = sum_{e: row=i} h_e_agg[col[e]].
  - Segment sums use a fixed "slot grid": each node gets C slots; edge k of node
    v goes to (block b = v//128, partition v%128, slot k). Grid slots are packed
    3-deep along the partition axis (3*33=99 rows) so one bf16 matmul per
    triple produces 3 slots' h_e pre-activations; a parallel accumulating
    matmul against stacked identities produces S_ea directly in PSUM.
    Nodes with deg > C spill to per-block overflow tiles handled with one-hot
    scatter matmuls (the one-hot matrices are host-precomputed).
  - h_e_agg is cast to bf16 and AllGather'd in 2 row-chunks, the first issued
    mid-phase-A so most of the collective overlaps compute. M is built from
    batched indirect-DMA gathers of h_e_agg[col] (one instruction per 7 blocks)
    in the same slot layout, then reduced along slots.
  - All matmul inputs are bf16 (PSUM accumulation stays f32); the final output
    is f32. Tolerance budget (2e-2 rel) dwarfs bf16 rounding (~4e-3).
"""

import numpy as np

import concourse.bass as bass
import concourse.mybir as mybir
import concourse.tile as tile
from concourse import bacc
from concourse._compat import get_trn_type
from concourse.bass import IndirectOffsetOnAxis
from concourse.bass_utils import run_bass_kernel_spmd

F32 = mybir.dt.float32
BF16 = mybir.dt.float16  # fp16: same PE speed as bf16, 8x mantissa
I32 = mybir.dt.int32
AX = mybir.AxisListType
OP = mybir.AluOpType
ACT = mybir.ActivationFunctionType

NPBF = mybir.dt.np(BF16)


class Cfg:
    def __init__(self, N=50000, E=800000, ncores=8, C=15, t_ov=3):
        self.N = N
        self.E = E
        self.ncores = ncores
        self.NPC = N // ncores            # nodes per core
        self.NBLK = (self.NPC + 127) // 128
        self.PADN = self.NBLK * 128       # padded nodes per core
        self.C = C                        # grid slots per node
        assert C % 3 == 0, "grid slots packed 3-deep along partitions"
        self.NTRI = C // 3                # packed triples per block
        self.t_ov = t_ov                  # overflow tiles per block
        # one 128-column eat group per overflow tile, all at base partition 0
        # (base-64 matmul operands crashed the device)
        self.NOVG = t_ov
        self.TPB = C + t_ov               # gather tiles per block
        self.EATW = (self.NTRI + self.NOVG) * 128   # eat columns per block
        self.ND = 128                     # node feature dim
        self.ED = 32                      # edge feature dim
        self.OUTD = 224
        # collective row-chunk boundaries (blocks): the first chunk's
        # AllGather is issued mid-phase-A so it finishes by phase-A end;
        # only the remainder is exposed (and phase C overlaps it).
        if self.NBLK > 18:
            self.CCH = (18, self.NBLK)
        elif self.NBLK > 1:
            self.CCH = (self.NBLK // 2, self.NBLK)
        else:
            self.CCH = (self.NBLK,)
        self.GCH = 7                      # blocks per indirect-gather instr

    def key(self):
        return (self.N, self.E, self.ncores, self.C, self.t_ov)


def build_program(cfg, skip=()):
    """Build the SPMD Bass program (same NEFF for all cores).

    skip: iterable of {"gather", "slotmm", "overflow", "phasec", "collective"} —
    timing-experiment knobs only (results are wrong when used)."""
    skip = set(skip)
    nc = bacc.Bacc(get_trn_type() or "TRN2", target_bir_lowering=False, debug=True)
    NBLK, TPB, C, t_ov, PADN = cfg.NBLK, cfg.TPB, cfg.C, cfg.t_ov, cfg.PADN
    NTRI, NOVG, EATW, GCH = cfg.NTRI, cfg.NOVG, cfg.EATW, cfg.GCH

    eat = nc.declare_dram_parameter("eat", [99, NBLK * EATW], BF16, isOutput=False)
    gidx = nc.declare_dram_parameter("gidx", [128, NBLK * TPB], I32, isOutput=False)
    oh3p = nc.declare_dram_parameter("oh3", [128, NBLK * t_ov * 128], BF16,
                                     isOutput=False)
    xt = nc.declare_dram_parameter("xt", [128, PADN], BF16, isOutput=False)
    xts = nc.declare_dram_parameter("xts", [128, PADN], BF16, isOutput=False)
    sdrow = nc.declare_dram_parameter("sdrow", [1, PADN], BF16, isOutput=False)
    dinvp = nc.declare_dram_parameter("dinv", [128, NBLK], F32, isOutput=False)
    rch = nc.declare_dram_parameter("rch", [99, 96], BF16, isOutput=False)
    rci = nc.declare_dram_parameter("rci", [99, 32], BF16, isOutput=False)
    rco = nc.declare_dram_parameter("rco", [33, 64], BF16, isOutput=False)
    wegot = nc.declare_dram_parameter("wegot", [128, 64], BF16, isOutput=False)
    wpxt = nc.declare_dram_parameter("wpxt", [128, 96], BF16, isOutput=False)
    wpet = nc.declare_dram_parameter("wpet", [32, 96], BF16, isOutput=False)
    bego = nc.declare_dram_parameter("bego", [1, 64], BF16, isOutput=False)
    bpeer = nc.declare_dram_parameter("bpeer", [1, 96], BF16, isOutput=False)
    ident = nc.declare_dram_parameter("ident", [128, 128], F32, isOutput=False)
    outp = nc.declare_dram_parameter("out", [PADN, cfg.OUTD], F32, isOutput=True)

    with tile.TileContext(nc) as tc:
        with (
            tc.tile_pool(name="const", bufs=1) as cp,
            tc.tile_pool(name="resident", bufs=1) as rp,
            tc.tile_pool(name="dram", bufs=1, space="DRAM") as dp,
        ):
            rch_sb = cp.tile([99, 96], BF16)
            nc.sync.dma_start(rch_sb[:], rch[:])
            rci_sb = cp.tile([99, 32], BF16)
            nc.sync.dma_start(rci_sb[:], rci[:])
            rco_sb = cp.tile([33, 64], BF16)
            nc.sync.dma_start(rco_sb[:], rco[:])
            wego_sb = cp.tile([128, 64], BF16)
            nc.sync.dma_start(wego_sb[:], wegot[:])
            wpx_sb = cp.tile([128, 96], BF16)
            nc.sync.dma_start(wpx_sb[:], wpxt[:])
            wpe_sb = cp.tile([32, 96], BF16)
            nc.sync.dma_start(wpe_sb[:], wpet[:])
            bego_sb = cp.tile([1, 64], BF16)
            nc.sync.dma_start(bego_sb[:], bego[:])
            bpeer_sb = cp.tile([1, 96], BF16)
            nc.sync.dma_start(bpeer_sb[:], bpeer[:])
            ident_sb = cp.tile([128, 128], F32)
            nc.sync.dma_start(ident_sb[:], ident[:])
            ones_sb = cp.tile([1, 128], BF16)
            nc.gpsimd.memset(ones_sb[:], 1.0)

            xt_sb = rp.tile([128, PADN], BF16)
            nc.sync.dma_start(xt_sb[:], xt[:])
            xts_sb = rp.tile([128, PADN], BF16)
            nc.sync.dma_start(xts_sb[:], xts[:])
            sdrow_sb = rp.tile([1, PADN], BF16)
            nc.sync.dma_start(sdrow_sb[:], sdrow[:])
            gidx_sb = rp.tile([128, NBLK * TPB], I32)
            nc.sync.dma_start(gidx_sb[:], gidx[:])
            dinv_sb = rp.tile([128, NBLK], F32)
            nc.sync.dma_start(dinv_sb[:], dinvp[:])
            oh3_sb = rp.tile([128, NBLK * t_ov * 128], BF16)
            half = NBLK * t_ov * 128 // 2
            nc.sync.dma_start(oh3_sb[:, 0:half], oh3p[:, 0:half])
            nc.sync.dma_start(oh3_sb[:, half:], oh3p[:, half:])

            heagg_bf = rp.tile([128, NBLK * 32], BF16)
            ts_sb = rp.tile([128, NBLK * 32], F32)
            outst_sb = rp.tile([128, NBLK * cfg.OUTD], F32)

            # collective chunking: each chunk has its own shared output tile
            # (shared DRAM admits one writer), bounced into one contiguous
            # local table zcat that the indirect gathers read.
            cbnds = [0] + [bb * 128 for bb in cfg.CCH]   # local-row boundaries
            zloc = dp.tile([PADN, 32], BF16)
            zags = [
                dp.tile([cfg.ncores * (cbnds[i + 1] - cbnds[i]), 32], BF16,
                        addr_space="Shared", name=f"zag{i}")
                for i in range(len(cfg.CCH))
            ]
            zcat = dp.tile([cfg.ncores * PADN, 32], BF16)

            # ---------------- Phase A: per-edge MLP + segment sums ----------
            with (
                tc.tile_pool(name="eatp", bufs=4) as eatp,
                tc.tile_pool(name="workA", bufs=6) as wp,
                tc.tile_pool(name="psA", bufs=2, space="PSUM") as psA,
                tc.tile_pool(name="psB", bufs=2, space="PSUM") as psB,
                tc.tile_pool(name="psP", bufs=2, space="PSUM") as psP,
                tc.tile_pool(name="psC", bufs=1, space="PSUM") as psC,
                tc.tile_pool(name="psT", bufs=1, space="PSUM") as psT,
            ):
                # A-core is software-pipelined: stage1(b) = DMA + independent
                # matmuls; stage2(b) = ops depending on stage1(b)'s PSUM,
                # emitted after stage1(b+1) so the in-order PE queue never
                # stalls on the overflow-extract round trip.
                def stage1(b):
                    ech = eatp.tile([99, EATW], BF16, tag="ech")
                    nc.sync.dma_start(ech[:], eat[:, b * EATW:(b + 1) * EATW])
                    bankA = psA.tile([128, 32], F32, tag="bankA")
                    bankB = psB.tile([128, C * 32], F32, tag="bankB")
                    # pc: per-overflow-edge [S_ea part | h_e part]; cols
                    # t_ov*64.. hold the scattered overflow h_e sums
                    pc = psP.tile([128, t_ov * 64 + 32], F32, tag="pc")
                    for o in ([] if "overflow" in skip else range(t_ov)):
                        lh = ech[0:33, (NTRI + o) * 128:(NTRI + o + 1) * 128]
                        nc.tensor.matmul(pc[:, o * 64:(o + 1) * 64], lh,
                                         rco_sb[:], start=True, stop=True)
                    for t in ([] if "slotmm" in skip else range(NTRI)):
                        lh = ech[:, t * 128:(t + 1) * 128]
                        nc.tensor.matmul(bankB[:, t * 96:(t + 1) * 96],
                                         lh, rch_sb[:], start=True, stop=True)
                        nc.tensor.matmul(bankA[:], lh, rci_sb[:],
                                         start=(t == 0), stop=False)
                    # overflow extraction: one copy + one relu over all tiles
                    ov_all = wp.tile([128, t_ov * 64], BF16, tag="ovall")
                    nc.vector.tensor_copy(ov_all[:], pc[:, 0:t_ov * 64])
                    ov_rel = wp.tile([128, t_ov * 64], BF16, tag="ovrel")
                    nc.scalar.activation(ov_rel[:], pc[:, 0:t_ov * 64], ACT.Relu)
                    return ech, bankA, bankB, pc, ov_all, ov_rel

                def stage2(b, st):
                    ech, bankA, bankB, pc, ov_all, ov_rel = st
                    ohe = pc[:, t_ov * 64:t_ov * 64 + 32]
                    for o in range(t_ov):
                        ohk = oh3_sb[:, (b * t_ov + o) * 128:(b * t_ov + o + 1) * 128]
                        nc.tensor.matmul(bankA[:], ohk,
                                         ov_all[:, o * 64:o * 64 + 32],
                                         start=False, stop=(o == t_ov - 1))
                        nc.tensor.matmul(ohe, ohk,
                                         ov_rel[:, o * 64 + 32:o * 64 + 64],
                                         start=(o == 0), stop=(o == t_ov - 1))
                    relu_st = wp.tile([128, C * 32], BF16, tag="relu")
                    nc.scalar.activation(relu_st[:], bankB[:], ACT.Relu)
                    t_he = wp.tile([128, 32], F32, tag="the")
                    nc.vector.tensor_reduce(
                        t_he[:],
                        relu_st[:].rearrange("p (j c) -> p c j", j=C),
                        axis=AX.X, op=OP.add,
                    )
                    nc.vector.tensor_tensor(
                        out=heagg_bf[:, b * 32:(b + 1) * 32],
                        in0=t_he[:], in1=ohe, op=OP.add)
                    # peer-branch S_ea scaled by deg^-1/2 (kept for phase C)
                    nc.vector.tensor_scalar_mul(ts_sb[:, b * 32:(b + 1) * 32],
                                                bankA[:], dinv_sb[:, b:b + 1])
                    ob = b * cfg.OUTD
                    nc.gpsimd.tensor_copy(outst_sb[:, ob + 64:ob + 96],
                                          heagg_bf[:, b * 32:(b + 1) * 32])

                def chunk_collective(ci):
                    blo = cbnds[ci] // 128
                    bhi = cfg.CCH[ci]
                    rlo, rhi = cbnds[ci], cbnds[ci + 1]
                    nc.sync.dma_start(
                        zloc[rlo:rhi, :].rearrange("(b p) c -> p b c", p=128),
                        heagg_bf[:, blo * 32:bhi * 32]
                        .rearrange("p (b c) -> p b c", c=32),
                    )
                    if "collective" not in skip:
                        nc.gpsimd.collective_compute(
                            "AllGather", OP.bypass,
                            ins=[zloc[rlo:rhi, :].opt()],
                            outs=[zags[ci][:].opt()],
                            replica_groups=[list(range(cfg.ncores))],
                        )
                        nc.sync.dma_start(
                            zcat[cfg.ncores * rlo:cfg.ncores * rhi, :], zags[ci][:])

                prev = None
                done = 0
                for b in range(NBLK):
                    st = stage1(b)
                    if prev is not None:
                        stage2(b - 1, prev)
                        if b in cfg.CCH:
                            chunk_collective(done)
                            done += 1
                    prev = st
                stage2(NBLK - 1, prev)
                chunk_collective(done)

                # ---- Phase C: node-level MLPs (overlaps the collectives) ----
                for b in ([] if "phasec" in skip else range(NBLK)):
                    ob = b * cfg.OUTD
                    pC = psC.tile([128, 160], F32, tag="pC")
                    nc.tensor.matmul(pC[:, 0:64], ones_sb[:], bego_sb[:],
                                     start=True, stop=False)
                    nc.tensor.matmul(pC[:, 0:64],
                                     xt_sb[:, b * 128:(b + 1) * 128], wego_sb[:],
                                     start=False, stop=True)
                    nc.vector.tensor_scalar_max(outst_sb[:, ob:ob + 64],
                                                pC[:, 0:64], 0.0)
                    # peer branch (fully accumulated in PSUM)
                    nc.tensor.matmul(pC[:, 64:160],
                                     sdrow_sb[:, b * 128:(b + 1) * 128],
                                     bpeer_sb[:], start=True, stop=False)
                    nc.tensor.matmul(pC[:, 64:160],
                                     xts_sb[:, b * 128:(b + 1) * 128], wpx_sb[:],
                                     start=False, stop=False)
                    pt = psT.tile([32, 128], F32, tag="pt")
                    nc.tensor.matmul(pt[:], ts_sb[:, b * 32:(b + 1) * 32],
                                     ident_sb[:], is_transpose=True,
                                     start=True, stop=True)
                    seat = wp.tile([32, 128], BF16, tag="seat")
                    nc.scalar.activation(seat[:], pt[:], ACT.Copy)
                    nc.tensor.matmul(pC[:, 64:160], seat[:], wpe_sb[:],
                                     start=False, stop=True)
                    nc.scalar.activation(outst_sb[:, ob + 128:ob + 224],
                                         pC[:, 64:160], ACT.Relu)

            # ------------- Phase B: gather/M + output assembly --------------
            with (
                tc.tile_pool(name="gp", bufs=2) as gp,
                tc.tile_pool(name="workB", bufs=4) as wb,
                tc.tile_pool(name="psM", bufs=4, space="PSUM") as psM,
            ):
                nsc = (NBLK + GCH - 1) // GCH
                for s in range(nsc):
                    b0 = s * GCH
                    b1 = min(b0 + GCH, NBLK)
                    nb = b1 - b0
                    g = gp.tile([128, GCH * TPB * 32], BF16, tag="g")
                    if "gather" in skip:
                        nc.vector.memset(g[:], 0.0)
                    else:
                        # HW DGE supports exactly one offset per partition per
                        # instruction (multi-column offset APs misbehave), so
                        # gather one slot-tile at a time.
                        for tt in range(nb * TPB):
                            nc.gpsimd.indirect_dma_start(
                                out=g[:, tt * 32:(tt + 1) * 32],
                                out_offset=None,
                                in_=zcat[:],
                                in_offset=IndirectOffsetOnAxis(
                                    ap=gidx_sb[:, b0 * TPB + tt:b0 * TPB + tt + 1],
                                    axis=0),
                            )
                    for b in range(b0, b1):
                        base = (b - b0) * TPB * 32
                        m_main = wb.tile([128, 32], F32, tag="mmain")
                        nc.vector.tensor_reduce(
                            m_main[:],
                            g[:, base:base + C * 32]
                            .rearrange("p (j c) -> p c j", j=C),
                            axis=AX.X, op=OP.add,
                        )
                        pm = psM.tile([128, 32], F32, tag="pm")
                        for o in range(t_ov):
                            ohk = oh3_sb[:, (b * t_ov + o) * 128:
                                         (b * t_ov + o + 1) * 128]
                            nc.tensor.matmul(
                                pm[:], ohk,
                                g[:, base + (C + o) * 32:base + (C + o + 1) * 32],
                                start=(o == 0), stop=(o == t_ov - 1))
                        t_m = wb.tile([128, 32], F32, tag="tm")
                        nc.vector.tensor_tensor(out=t_m[:], in0=m_main[:],
                                                in1=pm[:], op=OP.add)
                        ob = b * cfg.OUTD
                        nc.gpsimd.tensor_scalar_mul(outst_sb[:, ob + 96:ob + 128],
                                                    t_m[:], dinv_sb[:, b:b + 1])
                    # batched output store for this super-chunk
                    nc.sync.dma_start(
                        outp[b0 * 128:b1 * 128, :]
                        .rearrange("(b p) c -> p b c", p=128),
                        outst_sb[:, b0 * cfg.OUTD:b1 * cfg.OUTD]
                        .rearrange("p (b c) -> p b c", c=cfg.OUTD),
                    )
    nc.compile()
    return nc


def host_prep(cfg, x, edge_attrs, edge_index):
    """Shard + lay out inputs for the slot-grid kernel. Pure index work + O(N)
    scalar prep (degree normalizers); all O(E*H)/O(N*H) FP math runs on device."""
    N, E, C, NBLK, TPB, t_ov, NPC, PADN = (cfg.N, cfg.E, cfg.C, cfg.NBLK,
                                           cfg.TPB, cfg.t_ov, cfg.NPC, cfg.PADN)
    NTRI, NOVG, EATW = cfg.NTRI, cfg.NOVG, cfg.EATW
    row = np.asarray(edge_index[1]).astype(np.int64)
    col = np.asarray(edge_index[0]).astype(np.int64)
    ea = np.asarray(edge_attrs, dtype=np.float32)
    xf = np.asarray(x, dtype=np.float32)

    deg = np.bincount(row, minlength=N)
    degf = np.maximum(deg, 1).astype(np.float64)
    dinv = np.where(deg > 0, degf ** -0.5, 0.0).astype(np.float32)
    sqd = np.sqrt(deg.astype(np.float64)).astype(np.float32)

    core = row // NPC
    lrow = row - core * NPC
    blk = lrow // 128
    part = lrow % 128

    # rank of each edge within its destination node
    order = np.argsort(row, kind="stable")
    sorted_row = row[order]
    starts = np.searchsorted(sorted_row, np.arange(N), side="left")
    rank = np.empty(E, np.int64)
    rank[order] = np.arange(E) - starts[sorted_row]

    is_grid = rank < C
    ove = np.where(~is_grid)[0]
    ovkey = core[ove] * NBLK + blk[ove]
    o_order = np.argsort(ovkey, kind="stable")
    ove = ove[o_order]
    okey_sorted = ovkey[o_order]
    ostarts = np.searchsorted(okey_sorted, np.arange(NBLK * cfg.ncores), side="left")
    opos = np.arange(ove.size) - ostarts[okey_sorted]
    otile = opos // 128            # overflow tile index within block (0..t_ov)
    opart = opos % 128
    if ove.size and otile.max() >= t_ov:
        raise ValueError("overflow tiles exceeded; raise t_ov")

    # zag row layout: per collective chunk c (block boundaries cfg.CCH),
    # rows are [8*Rlo + r*(Rhi-Rlo) + (l-Rlo)] (AllGather core-major concat)
    cbounds = [0] + [bb * 128 for bb in cfg.CCH]   # local-row boundaries

    def zag_row(r, l):
        out = np.empty_like(l)
        for ci in range(len(cbounds) - 1):
            lo, hi = cbounds[ci], cbounds[ci + 1]
            m = (l >= lo) & (l < hi)
            out[m] = cfg.ncores * lo + r[m] * (hi - lo) + (l[m] - lo)
        return out

    assert NPC < PADN, "pad-slot gathers need a guaranteed-zero dummy row"
    zcore = col // NPC
    zl = col % NPC
    zrow = zag_row(zcore, zl)
    ZPAD = int(zag_row(np.array([0]), np.array([PADN - 1]))[0])  # core0 pad node

    in_maps = []
    for r in range(cfg.ncores):
        sel = core == r
        e_idx = np.where(sel)[0]
        blk_r = blk[e_idx]
        part_r = part[e_idx]
        rank_r = rank[e_idx]
        zrow_r = zrow[e_idx]
        ea_r = ea[e_idx]

        EAT = np.zeros((99, NBLK * EATW), np.float32)
        GIDX = np.full((128, NBLK * TPB), ZPAD, np.int32)
        OH3 = np.zeros((128, NBLK * t_ov * 128), np.float32)

        # grid edges: triple t = rank//3, band i = rank%3
        gm = rank_r < C
        gb, gp_, gr, gz = blk_r[gm], part_r[gm], rank_r[gm], zrow_r[gm]
        tri, band = gr // 3, gr % 3
        gcol = gb * EATW + tri * 128 + gp_
        for i in range(3):
            m = band == i
            EAT[33 * i:33 * i + 32, gcol[m]] = ea_r[gm][m].T
            EAT[33 * i + 32, gcol[m]] = 1.0
        GIDX[gp_, gb * TPB + gr] = gz.astype(np.int32)

        # overflow edges for this core
        om = ~gm
        sel_ov = (core[ove] == r)
        ov_r = ove[sel_ov]
        ot_r = otile[sel_ov]
        op_r = opart[sel_ov]
        ob_r = blk[ov_r]
        oz_r = zrow[ov_r]
        for o in range(t_ov):
            m = ot_r == o
            ocol = ob_r[m] * EATW + (NTRI + o) * 128 + op_r[m]
            EAT[0:32, ocol] = ea[ov_r[m]].T
            EAT[32, ocol] = 1.0
            GIDX[op_r[m], ob_r[m] * TPB + C + o] = oz_r[m].astype(np.int32)
            OH3[op_r[m], (ob_r[m] * t_ov + o) * 128 + part[ov_r[m]]] = 1.0

        lo, hi = r * NPC, (r + 1) * NPC
        XT = np.zeros((128, PADN), np.float32)
        XT[:, :NPC] = xf[lo:hi].T
        XTS = np.zeros((128, PADN), np.float32)
        XTS[:, :NPC] = (xf[lo:hi] * sqd[lo:hi, None]).T
        dl = np.zeros(PADN, np.float32)
        dl[:NPC] = dinv[lo:hi]
        sl = np.zeros(PADN, np.float32)
        sl[:NPC] = sqd[lo:hi]
        DINV = dl.reshape(NBLK, 128).T.copy()
        SDROW = sl.reshape(1, PADN)

        m = {
            "eat": EAT.astype(NPBF), "gidx": GIDX, "oh3": OH3.astype(NPBF),
            "xt": XT.astype(NPBF), "xts": XTS.astype(NPBF),
            "sdrow": SDROW.astype(NPBF), "dinv": DINV,
        }
        in_maps.append(m)
    return in_maps


def make_consts(cfg, W_peer, b_peer, W_ego, b_ego, W_edge, b_edge):
    We = np.asarray(W_edge, np.float32)
    be = np.asarray(b_edge, np.float32)
    RCH = np.zeros((99, 96), np.float32)
    RCI = np.zeros((99, 32), np.float32)
    for i in range(3):
        RCH[33 * i:33 * i + 32, 32 * i:32 * i + 32] = We.T
        RCH[33 * i + 32, 32 * i:32 * i + 32] = be
        RCI[33 * i:33 * i + 32, :] = np.eye(32, dtype=np.float32)
    RCO = np.zeros((33, 64), np.float32)
    RCO[:32, :32] = np.eye(32, dtype=np.float32)
    RCO[:32, 32:64] = We.T
    RCO[32, 32:64] = be
    consts = {
        "rch": RCH.astype(NPBF), "rci": RCI.astype(NPBF), "rco": RCO.astype(NPBF),
        "wegot": np.ascontiguousarray(np.asarray(W_ego, np.float32).T).astype(NPBF),
        "wpxt": np.ascontiguousarray(
            np.asarray(W_peer, np.float32)[:, :128].T).astype(NPBF),
        "wpet": np.ascontiguousarray(
            np.asarray(W_peer, np.float32)[:, 128:].T).astype(NPBF),
        "bego": np.asarray(b_ego, np.float32).reshape(1, 64).astype(NPBF),
        "bpeer": np.asarray(b_peer, np.float32).reshape(1, 96).astype(NPBF),
        "ident": np.eye(128, dtype=np.float32),
    }
    return consts


_CACHE = {}
RUN_KWARGS = {}


def kernel(x, edge_attrs, W_peer, b_peer, W_ego, b_ego, W_edge, b_edge, edge_index):
    x = np.asarray(x)
    edge_attrs = np.asarray(edge_attrs)
    edge_index = np.asarray(edge_index)
    N, E = x.shape[0], edge_attrs.shape[0]

    # pick t_ov from the actual degree distribution (>=3 keeps NEFF cache warm
    # for the expected data)
    row = edge_index[1].astype(np.int64)
    C = 15
    ncores = 8
    NPC = N // ncores
    NBLK = (NPC + 127) // 128
    deg = np.bincount(row, minlength=N)
    ovn = np.maximum(deg - C, 0)
    nodes = np.arange(N)
    bkey = (nodes // NPC) * NBLK + (nodes % NPC) // 128
    ovblk = np.bincount(bkey, weights=ovn.astype(np.float64), minlength=NBLK * ncores)
    t_ov = max(3, int(np.ceil(ovblk.max() / 128.0)))

    cfg = Cfg(N=N, E=E, ncores=ncores, C=C, t_ov=t_ov)
    key = cfg.key()
    if key not in _CACHE:
        _CACHE[key] = build_program(cfg)
    nc = _CACHE[key]

    in_maps = host_prep(cfg, x, edge_attrs, edge_index)
    consts = make_consts(cfg, W_peer, b_peer, W_ego, b_ego, W_edge, b_edge)
    for m in in_maps:
        m.update(consts)

    res = run_bass_kernel_spmd(nc, in_maps, core_ids=list(range(cfg.ncores)),
                               **RUN_KWARGS)
    out = np.empty((N, cfg.OUTD), np.float32)
    for r in range(cfg.ncores):
        out[r * cfg.NPC:(r + 1) * cfg.NPC] = res.results[r]["out"][:cfg.NPC]
    if RUN_KWARGS:
        kernel.last_result = res
    return out
